# revision 1
# baseline (speedup 1.0000x reference)
"""Multi-head differential attention Trainium2 kernel (8 NeuronCores).

Sharding: core c -> batch b = c // 4, head group g = c % 4 (4 of 16 heads).

Sparsity: the mask zeroes whole QUERY rows; after softmax those rows carry
uniform attention, so their output is one shared vector per (batch, head).
The host compacts the active query rows (1046 of 2048 for the shipped mask)
to the front, the device computes attention only for ceil(active/128)*128
rows, and the host broadcasts the single "masked row" result (computed in
numpy from column means) into the masked positions.

Math notes (same as the dense predecessor):
 - Softmax normalization is deferred: LN is invariant to positive row scale,
   so we feed LN with y'' = r2*y1 - (lam*r1)*y2 where r1/r2 are exp-row-sums.
 - The trailing (1 - lambda_init) factor is folded into ln_w / ln_b.
 - The 1/sqrt(HS) score scale is folded into Wq host-side.
 - Partition-dim reductions (row sums r, LN mean/var) are done with a
   ones[128,128] matmul on the tensor engine instead of gpsimd.
"""

import math
import os
import sys

sys.path.insert(0, "/opt/trn_rl_repo")

# Engine-usage toggles.  The conservative defaults (all DMA on the SP HWDGE
# ring, epilogue element-wise ops on DVE rather than gpsimd) are both faster
# in the cost model and match the engine paths proven on this runtime.
SP_ONLY_DMA = os.environ.get("KOPT_ACTDMA") != "1"
NO_POOL = os.environ.get("KOPT_POOL") != "1"

import ml_dtypes
import numpy as np

import concourse.bass as bass
import concourse.bass_isa as bass_isa
import concourse.mybir as mybir
from concourse import bacc
from concourse.bass import ds, ts
from concourse.bass_utils import run_bass_kernel_spmd
from concourse.tile import TileContext

B, T, C, H = 2, 2048, 1024, 16
HS = C // H            # 64
D2 = 2 * HS            # 128
LAYER_IDX = 2
LAMBDA_INIT = 0.8 - 0.6 * float(np.exp(-0.3 * (LAYER_IDX - 1)))
EPS = 1e-9
N_CORES = 8
GPB = N_CORES // B          # core groups per batch = 4
HPC = H // GPB              # heads per core = 4
NKT = T // 128              # 16 k tiles
DEFAULT_NQA = 12            # 1046 active rows -> 1536 (rounded to 512-multiples)
DEFAULT_NQ_OUT = 9          # out-projection tiles actually read (ceil(1046/128))

FP32 = mybir.dt.float32
BF16 = mybir.dt.bfloat16
AF = mybir.ActivationFunctionType
ALU = mybir.AluOpType

_CACHED = {}


def _q_blocks(ta):
    """Split the active-q extent into blocks of <=512 (PSUM-bank-legal sizes).

    Matmul outputs may not cross a 2KB PSUM bank boundary, so the two score
    maps laid out side by side at [0:sz] and [sz:2sz] require sz in
    {128, 256, 512}: either both maps in bank 0 (sz<=256) or exactly
    bank-aligned (sz=512).
    """
    out, off = [], 0
    while off < ta:
        rem = ta - off
        sz = 512 if rem >= 512 else (256 if rem >= 256 else 128)
        out.append((off, sz))
        off += sz
    return out


def build_nc(repeat=1, mode='all', nqa=None, nq_out=None):
    nqa = DEFAULT_NQA if nqa is None else nqa
    nq_out = DEFAULT_NQ_OUT if nq_out is None else min(nq_out, nqa)
    TA = nqa * 128
    TP = -(-TA // 512) * 512          # transpose/DMA extent, 512-multiple
    blocks = _q_blocks(TA)

    nc = bacc.Bacc("TRN2", target_bir_lowering=False, debug=False,
                   enable_asserts=False)

    xq_d = nc.dram_tensor("xq", [TP, C], BF16, kind="ExternalInput").ap()
    xk_d = nc.dram_tensor("xk", [T, C], BF16, kind="ExternalInput").ap()
    xv_d = nc.dram_tensor("xv", [T, C], BF16, kind="ExternalInput").ap()
    # weights, host packed to SBUF layout (partition dim first)
    wq_d = nc.dram_tensor("wq", [128, HPC * 8 * 128], BF16, kind="ExternalInput").ap()
    wk_d = nc.dram_tensor("wk", [128, HPC * 8 * 128], BF16, kind="ExternalInput").ap()
    wv_d = nc.dram_tensor("wv", [128, 8 * 512], BF16, kind="ExternalInput").ap()
    wc_d = nc.dram_tensor("wc", [128, HPC * 1024], BF16, kind="ExternalInput").ap()
    lnw_d = nc.dram_tensor("lnw", [128, 1], FP32, kind="ExternalInput").ap()
    lnb_d = nc.dram_tensor("lnb", [128, 1], FP32, kind="ExternalInput").ap()
    lq1_d = nc.dram_tensor("lq1", [1, HPC * HS], FP32, kind="ExternalInput").ap()
    lk1_d = nc.dram_tensor("lk1", [1, HPC * HS], FP32, kind="ExternalInput").ap()
    lq2_d = nc.dram_tensor("lq2", [1, HPC * HS], FP32, kind="ExternalInput").ap()
    lk2_d = nc.dram_tensor("lk2", [1, HPC * HS], FP32, kind="ExternalInput").ap()
    out_d = nc.dram_tensor("out", [TA, C], FP32, kind="ExternalOutput").ap()

    gp = nc.vector if NO_POOL else nc.gpsimd

    with TileContext(nc) as tc:
      for _rep in range(repeat):
        with (
            tc.tile_pool(name="singles", bufs=1) as singles,
            tc.tile_pool(name="maps", bufs=1) as maps,
        ):
            # ---------- constants / tiny prep ----------
            lnw_sb = singles.tile([128, 1], FP32, tag="lnw")
            lnb_sb = singles.tile([128, 1], FP32, tag="lnb")
            nc.sync.dma_start(out=lnw_sb, in_=lnw_d)
            nc.sync.dma_start(out=lnb_sb, in_=lnb_d)

            # lambda per head: lam = exp(sum(lq1*lk1)) - exp(sum(lq2*lk2)) + l0
            lrow = singles.tile([1, HPC * HS], FP32, tag="lrow")
            lrow2 = singles.tile([1, HPC * HS], FP32, tag="lrow2")
            ltmp = singles.tile([1, HPC * HS], FP32, tag="ltmp")
            s1 = singles.tile([1, HPC], FP32, tag="s1")
            s2 = singles.tile([1, HPC], FP32, tag="s2")
            lam_row = singles.tile([1, HPC], FP32, tag="lam_row")
            nc.sync.dma_start(out=lrow, in_=lq1_d)
            nc.sync.dma_start(out=lrow2, in_=lk1_d)
            nc.vector.tensor_mul(ltmp, lrow, lrow2)
            nc.vector.reduce_sum(s1, ltmp.rearrange("p (h d) -> p h d", d=HS),
                                 axis=mybir.AxisListType.X)
            nc.sync.dma_start(out=lrow, in_=lq2_d)
            nc.sync.dma_start(out=lrow2, in_=lk2_d)
            nc.vector.tensor_mul(ltmp, lrow, lrow2)
            nc.vector.reduce_sum(s2, ltmp.rearrange("p (h d) -> p h d", d=HS),
                                 axis=mybir.AxisListType.X)
            nc.scalar.activation(s1, s1, AF.Exp)
            nc.scalar.activation(s2, s2, AF.Exp)
            nc.vector.tensor_sub(lam_row, s1, s2)
            nc.vector.tensor_scalar_add(lam_row, lam_row, LAMBDA_INIT)
            lam_col = singles.tile([128, HPC], FP32, tag="lam_col")
            nc.gpsimd.partition_broadcast(lam_col, lam_row, 128)

            eps_col = singles.tile([128, 1], FP32, tag="eps_col")
            nc.vector.memset(eps_col, EPS)
            ones_sb = singles.tile([128, 128], BF16, tag="ones")
            nc.vector.memset(ones_sb, 1.0)

            wc_sb = singles.tile([128, HPC * 1024], BF16, tag="wc")
            nc.sync.dma_start(out=wc_sb, in_=wc_d)

            # ---------- transposed x loads + projections ----------
            qmapT = [maps.tile([128, TA], BF16, tag=f"qm{h}", name=f"qm{h}")
                     for h in range(HPC)]
            kmapT = [maps.tile([128, T], BF16, tag=f"km{h}", name=f"km{h}")
                     for h in range(HPC)]
            vv = [maps.tile([128, 4 * D2], BF16, tag=f"vv{i}", name=f"vv{i}")
                  for i in range(NKT)]
            ynormT = [maps.tile([128, TA], BF16, tag=f"yn{h}", name=f"yn{h}")
                      for h in range(HPC)]

            with (
                tc.tile_pool(name="wpool", bufs=1) as wpool,
                tc.tile_pool(name="xt", bufs=10) as xt_pool,
                tc.tile_pool(name="ppsum", bufs=4, space="PSUM") as ppsum,
            ):
                wq_sb = wpool.tile([128, HPC * 8 * 128], BF16, tag="wq")
                wk_sb = wpool.tile([128, HPC * 8 * 128], BF16, tag="wk")
                wv_sb = wpool.tile([128, 8 * 512], BF16, tag="wv")
                nc.sync.dma_start(out=wv_sb, in_=wv_d)
                nc.sync.dma_start(out=wk_sb, in_=wk_d)
                nc.sync.dma_start(out=wq_sb, in_=wq_d)

                def w_qk(w_sb, h, ct):   # [128, 128] lhsT (C-tile ct, head h)
                    return w_sb[:, ds((h * 8 + ct) * 128, 128)]

                def load_xt(x_d, nm, cols):
                    # Chunk each transpose into 512-row pieces so consumers
                    # start as soon as their rows land, and alternate the
                    # SP / Activation HWDGE queues so the two DMA rings run
                    # in parallel on hardware.
                    tiles = [xt_pool.tile([128, T], BF16, tag="xt",
                                          name=f"{nm}{i}") for i in range(8)]
                    for c0 in range(0, cols, 512):
                        cw = min(512, cols - c0)
                        for i in range(8):
                            eng = nc.sync if (SP_ONLY_DMA or i % 2 == 0) \
                                else nc.scalar
                            eng.dma_start_transpose(
                                tiles[i][:, ds(c0, cw)],
                                x_d[ds(c0, cw), ds(i * 128, 128)])
                    return tiles

                xvT = load_xt(xv_d, "xv", T)
                for kt in range(NKT):
                    ps = ppsum.tile([128, 512], FP32, tag="ppsum", name="pp")
                    for ct in range(8):
                        nc.tensor.matmul(ps, xvT[ct][:, ds(kt * 128, 128)],
                                         wv_sb[:, ds(ct * 512, 512)],
                                         start=(ct == 0), stop=(ct == 7))
                    nc.vector.tensor_copy(vv[kt], ps)

                xkT = load_xt(xk_d, "xk", T)
                for h in range(HPC):
                    for kb in range(T // 512):
                        ps = ppsum.tile([128, 512], FP32, tag="ppsum", name="pp")
                        for ct in range(8):
                            nc.tensor.matmul(ps, w_qk(wk_sb, h, ct),
                                             xkT[ct][:, ds(kb * 512, 512)],
                                             start=(ct == 0), stop=(ct == 7))
                        nc.scalar.activation(kmapT[h][:, ds(kb * 512, 512)],
                                             ps, AF.Copy)

                xqT = load_xt(xq_d, "xq", TP)
                for h in range(HPC):
                    for (off, sz) in blocks:
                        ps = ppsum.tile([128, 512], FP32, tag="ppsum", name="pp")
                        for ct in range(8):
                            nc.tensor.matmul(ps[:, 0:sz], w_qk(wq_sb, h, ct),
                                             xqT[ct][:, ds(off, sz)],
                                             start=(ct == 0), stop=(ct == 7))
                        nc.scalar.activation(qmapT[h][:, ds(off, sz)],
                                             ps[:, 0:sz], AF.Copy)

            # ---------- attention ----------
            # Blocks are lists of "lanes" (h, qoff, sz, coff): full 512-wide
            # blocks have one lane; the <=128 remainder tiles of all 4 heads
            # are fused into ONE tail block (4 lanes) so the kt loop keeps a
            # uniform ~1us rhythm instead of running overhead-bound on tiny
            # per-head tail blocks.  Column layout within the shared [128,1024]
            # psum/sbuf tiles: lane maps sit at [coff, coff+sz) and
            # [c2off, c2off+sz) with c2off chosen so no matmul output crosses
            # a 2KB PSUM bank.
            full_per_head = [(off, sz) for (off, sz) in blocks if sz == 512]
            tail = [(off, sz) for (off, sz) in blocks if sz != 512]
            blk_list = []
            for h in range(HPC):
                for (off, sz) in full_per_head:
                    blk_list.append([(h, off, sz, 0)])
            if tail:
                toff, tsz = tail[0]
                if tsz == 128 and len(tail) == 1:
                    blk_list.append([(h, toff, 128, h * 256)
                                     for h in range(HPC)])
                else:
                    for h in range(HPC):
                        for (off, sz) in tail:
                            blk_list.append([(h, off, sz, 0)])

            def lane_cols(lanes):
                tot = 0
                for (h, off, sz, coff) in lanes:
                    tot = max(tot, coff + 2 * sz if sz < 512 else 1024)
                return tot

            def c2off(sz, coff):
                return coff + (512 if sz == 512 else sz)

            with (
                tc.tile_pool(name="escr", bufs=17) as e_pool,
                tc.tile_pool(name="scr", bufs=2) as scr_pool,
                tc.tile_pool(name="spsum", bufs=2, space="PSUM") as spsum,
                tc.tile_pool(name="ypsum", bufs=1, space="PSUM") as ypsum,
                tc.tile_pool(name="epsum", bufs=1, space="PSUM") as epsum,
            ):
                def emit_s(lanes, kt):
                    ksl = ds(kt * 128, 128)
                    s = spsum.tile([128, 2 * 512], FP32, tag="s", name="s")
                    for (h, off, sz, coff) in lanes:
                        qsl = ds(off, sz)
                        nc.tensor.matmul(s[:, ds(coff, sz)],
                                         kmapT[h][0:64, ksl],
                                         qmapT[h][0:64, qsl],
                                         start=True, stop=True,
                                         tile_position=(0, 0))
                        nc.tensor.matmul(s[:, ds(c2off(sz, coff), sz)],
                                         kmapT[h][64:128, ksl],
                                         qmapT[h][64:128, qsl],
                                         start=True, stop=True,
                                         tile_position=(64, 0))
                    return s

                def emit_deferred_a(p):
                    ylnq, tot = p["ylnq"], p["tot"]
                    stats = epsum.tile([128, 2 * 512], FP32, tag="ep",
                                       name="stats")
                    for c0 in range(0, tot, 512):
                        cw = min(512, tot - c0)
                        nc.tensor.matmul(stats[:, ds(c0, cw)], ones_sb,
                                         ylnq[:, ds(c0, cw)],
                                         start=True, stop=True)
                    mv = scr_pool.tile([128, 2 * 512], FP32, tag="mv",
                                       name="mv")
                    nc.vector.tensor_scalar(mv[:, 0:tot], stats[:, 0:tot],
                                            1.0 / D2, None, op0=ALU.mult)
                    for (h, off, sz, coff) in p["lanes"]:
                        mean = mv[:, ds(coff, sz)]
                        var = mv[:, ds(c2off(sz, coff), sz)]
                        msq = scr_pool.tile([128, 512], FP32, tag="msq",
                                            name="msq")[:, 0:sz]
                        gp.tensor_mul(msq, mean, mean)
                        gp.tensor_sub(var, var, msq)
                    p["mv"] = mv

                def emit_deferred_b(p):
                    mv = p["mv"]
                    for (h, off, sz, coff) in p["lanes"]:
                        qsl = ds(off, sz)
                        yln = p["ylnq"][:, ds(coff, sz)]
                        mean = mv[:, ds(coff, sz)]
                        var = mv[:, ds(c2off(sz, coff), sz)]
                        # rstd = exp(-0.5 * ln(var + eps))
                        nc.scalar.activation(var, var, AF.Ln, bias=eps_col)
                        nc.scalar.activation(var, var, AF.Exp, scale=-0.5)
                        tno = scr_pool.tile([128, 512], FP32, tag="tno",
                                            name="tno")[:, 0:sz]
                        gp.tensor_sub(tno, yln, mean)
                        gp.tensor_mul(tno, tno, var)
                        nc.vector.tensor_scalar(ynormT[h][:, qsl], tno,
                                                lnw_sb, lnb_sb,
                                                op0=ALU.mult, op1=ALU.add)

                pend = None
                s_carry = None
                for bi, lanes in enumerate(blk_list):
                    tot = lane_cols(lanes)
                    y12 = ypsum.tile([128, 2 * 512], FP32, tag="y", name="y12")
                    ra0 = scr_pool.tile([128, 2 * 512], BF16, tag="ra0", name="ra0")
                    ra1 = scr_pool.tile([128, 2 * 512], BF16, tag="ra1", name="ra1")
                    fused = len(lanes) > 1
                    e_keep = []
                    s_next = s_carry if s_carry is not None else emit_s(lanes, 0)
                    s_carry = None
                    for kt in range(NKT):
                        s = s_next
                        if kt + 1 < NKT:
                            s_next = emit_s(lanes, kt + 1)
                        elif bi + 1 < len(blk_list):
                            s_carry = emit_s(blk_list[bi + 1], 0)
                        if kt < 2 and not fused:
                            e = (ra0 if kt == 0 else ra1)[:, 0:tot]
                        else:
                            e = e_pool.tile([128, 2 * 512], BF16,
                                            tag="e", name="e")[:, 0:tot]
                        nc.scalar.activation(e, s[:, 0:tot], AF.Exp)
                        if fused:
                            e_keep.append(e)
                        # inline y accumulation: each open psum group must own
                        # its 2KB bank, so a fused block only accumulates
                        # lanes 0 and 2 (banks 0/1) inline; lanes 1/3 run as
                        # post-loop passes over the retained e tiles.
                        for li, (h, off, sz, coff) in enumerate(lanes):
                            hsl = ds(h * D2, D2)
                            if sz == 512:
                                nc.tensor.matmul(y12[:, 0:512],
                                                 vv[kt][:, hsl], e[:, 0:512],
                                                 start=(kt == 0),
                                                 stop=(kt == NKT - 1))
                                nc.tensor.matmul(y12[:, ds(512, 512)],
                                                 vv[kt][:, hsl],
                                                 e[:, ds(512, 512)],
                                                 start=(kt == 0),
                                                 stop=(kt == NKT - 1))
                            elif not fused or li % 2 == 0:
                                nc.tensor.matmul(y12[:, ds(coff, 2 * sz)],
                                                 vv[kt][:, hsl],
                                                 e[:, ds(coff, 2 * sz)],
                                                 start=(kt == 0),
                                                 stop=(kt == NKT - 1))
                        if kt < 2 and fused:
                            nc.vector.tensor_copy(
                                (ra0 if kt == 0 else ra1)[:, 0:tot], e)
                        if kt >= 2:
                            tgt = (ra0 if kt % 2 == 0 else ra1)[:, 0:tot]
                            nc.vector.tensor_add(tgt, tgt, e)
                        if kt == 4 and pend is not None:
                            emit_deferred_a(pend)
                        if kt == 10 and pend is not None:
                            emit_deferred_b(pend)
                            pend = None
                    for li, (h, off, sz, coff) in enumerate(lanes):
                        if not fused or li % 2 == 0:
                            continue
                        hsl = ds(h * D2, D2)
                        for kt in range(NKT):
                            nc.tensor.matmul(y12[:, ds(coff, 2 * sz)],
                                             vv[kt][:, hsl],
                                             e_keep[kt][:, ds(coff, 2 * sz)],
                                             start=(kt == 0),
                                             stop=(kt == NKT - 1))

                    # ---- quick epilogue: drain PSUM, prep LN inputs ----
                    rsum = scr_pool.tile([128, 2 * 512], BF16, tag="rsum", name="rsum")
                    nc.vector.tensor_add(rsum[:, 0:tot], ra0[:, 0:tot],
                                         ra1[:, 0:tot])
                    rall = epsum.tile([128, 2 * 512], FP32, tag="ep",
                                      name="rall")
                    for c0 in range(0, tot, 512):
                        cw = min(512, tot - c0)
                        nc.tensor.matmul(rall[:, ds(c0, cw)], ones_sb,
                                         rsum[:, ds(c0, cw)],
                                         start=True, stop=True)
                    ylnq = scr_pool.tile([128, 2 * 512], BF16, tag="ylnq", name="ylnq")
                    for (h, off, sz, coff) in lanes:
                        r1 = rall[:, ds(coff, sz)]
                        r2 = rall[:, ds(c2off(sz, coff), sz)]
                        y1h = scr_pool.tile([128, 512], FP32, tag="y1h", name="y1h")[:, 0:sz]
                        nc.vector.tensor_copy(y1h, y12[:, ds(coff, sz)])
                        c2 = scr_pool.tile([128, 512], FP32, tag="c2", name="c2")[:, 0:sz]
                        nc.vector.tensor_scalar(c2, r1, lam_col[:, ds(h, 1)],
                                                None, op0=ALU.mult)
                        nc.vector.tensor_mul(y1h, y1h, r2)
                        nc.vector.tensor_mul(c2, c2,
                                             y12[:, ds(c2off(sz, coff), sz)])
                        yln = ylnq[:, ds(coff, sz)]
                        ysq = ylnq[:, ds(c2off(sz, coff), sz)]
                        nc.vector.tensor_sub(yln, y1h, c2)
                        nc.vector.tensor_mul(ysq, yln, yln)
                    pend = dict(lanes=lanes, ylnq=ylnq, tot=tot)
                emit_deferred_a(pend)
                emit_deferred_b(pend)

            # ---------- output projection ----------
            with (
                tc.tile_pool(name="obuf", bufs=2) as ob_pool,
                tc.tile_pool(name="opsum", bufs=4, space="PSUM") as opsum,
            ):
                for qt_i in range(nq_out):
                    qsl = ds(qt_i * 128, 128)
                    ob = ob_pool.tile([128, C], FP32, tag="ob", name="ob")
                    for cs in range(2):
                        ps = opsum.tile([128, 512], FP32, tag="op", name="op")
                        for h in range(HPC):
                            nc.tensor.matmul(ps, ynormT[h][:, qsl],
                                             wc_sb[:, ds(h * 1024 + cs * 512, 512)],
                                             start=(h == 0), stop=(h == HPC - 1))
                        nc.vector.tensor_copy(ob[:, ds(cs * 512, 512)], ps)
                    nc.sync.dma_start(out=out_d[qsl, :], in_=ob)

    # Force every activation (Exp + Ln + Copy + Square) onto the combined
    # natural_log_exp_and_others table set so no ACT_TABLE_LOADs are emitted
    # mid-kernel (~2.7us per switch).
    _orig_tables = bacc.get_activation_tables

    def _only_combined(arch):
        out = {}
        for name, funcs in _orig_tables(arch).items():
            out[name] = funcs if name == "natural_log_exp_and_others" else set()
        return out

    bacc.get_activation_tables = _only_combined
    try:
        nc.compile()
    finally:
        bacc.get_activation_tables = _orig_tables
    return nc


def _nqa_for(mask):
    # Round the active-row extent up to a multiple of 512 (4 tiles): the
    # 512-wide q-blocks are the only shapes validated end-to-end on this
    # runtime (non-512-multiple extents fail at NEFF execution).
    counts = (np.asarray(mask) != 0).sum(axis=1)
    nqa = max(1, int(math.ceil(counts.max() / 128.0)))
    return min(T // 128, -(-nqa // 4) * 4)


def _active_sel(mask_b, ta):
    """Active row indices first, padded with masked rows up to ta."""
    act = np.nonzero(mask_b != 0)[0]
    msk = np.nonzero(mask_b == 0)[0]
    pad = ta - len(act)
    if pad > 0:
        sel = np.concatenate([act, msk[:pad]])
    else:
        sel = act[:ta]
    return sel, min(len(act), ta)


def _prep_core_inputs(inputs, core, nqa=None):
    if nqa is None:
        nqa = int(os.environ.get("KOPT_NQA", "0")) or _nqa_for(inputs["mask"])
    ta = nqa * 128
    tp = -(-ta // 512) * 512
    b = core // GPB
    g = core % GPB
    h2 = slice(g * HPC * D2, (g + 1) * HPC * D2)          # 128/head cols
    bf = ml_dtypes.bfloat16
    sel, _ = _active_sel(np.asarray(inputs["mask"])[b], tp)

    qscale = np.float32(1.0 / math.sqrt(HS))

    def pack_qk(w1, w2, scale):
        # -> [128, HPC*8*128]: per head the 8 C-tiles of [W1_h | W2_h]
        cols = []
        for h in range(HPC):
            hh = slice((g * HPC + h) * HS, (g * HPC + h + 1) * HS)
            w = np.concatenate([w1[:, hh], w2[:, hh]], axis=1) * scale
            cols.append(w.reshape(8, 128, 128))
        arr = np.stack(cols, 0)                    # [HPC, 8, 128, 128]
        return np.ascontiguousarray(
            arr.transpose(2, 0, 1, 3).reshape(128, -1)).astype(bf)

    wv = inputs["Wv"][:, h2].reshape(8, 128, HPC * D2)
    wv = np.ascontiguousarray(wv.transpose(1, 0, 2).reshape(128, -1)).astype(bf)
    wc = inputs["Wc"][h2, :].reshape(HPC, 128, C)
    wc = np.ascontiguousarray(wc.transpose(1, 0, 2).reshape(128, -1)).astype(bf)

    sc = np.float32(1.0 - LAMBDA_INIT)
    heads = slice(g * HPC, (g + 1) * HPC)
    return {
        "xq": inputs["q"][b][sel].astype(bf),
        "xk": inputs["k"][b].astype(bf),
        "xv": inputs["v"][b].astype(bf),
        "wq": pack_qk(inputs["Wq1"], inputs["Wq2"], qscale),
        "wk": pack_qk(inputs["Wk1"], inputs["Wk2"], np.float32(1.0)),
        "wv": wv,
        "wc": wc,
        "lnw": (inputs["ln_w"] * sc).astype(np.float32).reshape(128, 1),
        "lnb": (inputs["ln_b"] * sc).astype(np.float32).reshape(128, 1),
        "lq1": inputs["lq1"][heads].astype(np.float32).reshape(1, -1),
        "lk1": inputs["lk1"][heads].astype(np.float32).reshape(1, -1),
        "lq2": inputs["lq2"][heads].astype(np.float32).reshape(1, -1),
        "lk2": inputs["lk2"][heads].astype(np.float32).reshape(1, -1),
    }


def _masked_row(inputs, b):
    """Output row (minus bc) for a masked query position: uniform attention."""
    f64 = np.float64
    lam = (np.exp((inputs["lq1"].astype(f64) * inputs["lk1"]).sum(-1))
           - np.exp((inputs["lq2"].astype(f64) * inputs["lk2"]).sum(-1))
           + LAMBDA_INIT)                                    # [H]
    vbar = inputs["v"][b].astype(f64).mean(0) @ inputs["Wv"].astype(f64) \
        + inputs["bv"].astype(f64)                           # [2C]
    yh = vbar.reshape(H, D2) * (1.0 - lam)[:, None]          # [H, 2HS]
    u = yh.mean(-1, keepdims=True)
    s = ((yh - u) ** 2).mean(-1, keepdims=True)
    yn = inputs["ln_w"].astype(f64) * (yh - u) / np.sqrt(s + 1e-12) \
        + inputs["ln_b"].astype(f64)
    yn = yn * (1.0 - LAMBDA_INIT)
    return (yn.reshape(2 * C) @ inputs["Wc"].astype(f64)).astype(np.float32)


def kernel(q, k, v, mask, Wq1, bq1, Wq2, bq2, Wk1, bk1, Wk2, bk2,
           Wv, bv, Wc, bc, ln_w, ln_b, lq1, lk1, lq2, lk2, **run_kw):
    inputs = dict(q=np.asarray(q), k=np.asarray(k), v=np.asarray(v),
                  mask=np.asarray(mask), Wq1=np.asarray(Wq1),
                  Wq2=np.asarray(Wq2), Wk1=np.asarray(Wk1), Wk2=np.asarray(Wk2),
                  Wv=np.asarray(Wv), bv=np.asarray(bv), Wc=np.asarray(Wc),
                  ln_w=np.asarray(ln_w), ln_b=np.asarray(ln_b),
                  lq1=np.asarray(lq1), lk1=np.asarray(lk1),
                  lq2=np.asarray(lq2), lk2=np.asarray(lk2))
    nqa = int(os.environ.get("KOPT_NQA", "0")) or _nqa_for(inputs["mask"])
    ta = nqa * 128
    counts = (inputs["mask"] != 0).sum(axis=1)
    nq_out = min(nqa, max(1, -(-int(counts.max()) // 128)))
    key = ("nc", nqa, nq_out)
    if key not in _CACHED:
        _CACHED[key] = build_nc(nqa=nqa, nq_out=nq_out)
    nc = _CACHED[key]
    in_maps = [_prep_core_inputs(inputs, c, nqa) for c in range(N_CORES)]
    res = run_bass_kernel_spmd(nc, in_maps, list(range(N_CORES)), **run_kw)
    _CACHED["last_results"] = res

    out = np.zeros((B, T, C), np.float32)
    for b in range(B):
        acc = np.zeros((ta, C), np.float32)
        for gcore in range(GPB):
            acc += np.asarray(
                res.results[b * GPB + gcore]["out"]).astype(np.float32)
        sel, nact = _active_sel(inputs["mask"][b], ta)
        out[b][sel[:nact]] = acc[:nact]
        mrow = _masked_row(inputs, b)
        out[b][inputs["mask"][b] == 0] = mrow[None, :]
    out += np.asarray(bc, np.float32)[None, None, :]
    return out



# revision 5
# speedup vs baseline: 4.8344x; 4.8344x over previous
"""Multi-head differential attention Trainium2 kernel (8 NeuronCores).

Sharding: core c -> batch b = c // 4, head group g = c % 4 (4 of 16 heads).

Sparsity: the mask zeroes whole QUERY rows; after softmax those rows carry
uniform attention, so their output is one shared vector per (batch, head).
The host compacts the active query rows (1046 of 2048 for the shipped mask)
to the front, the device computes attention only for ceil(active/128)*128
rows, and the host broadcasts the single "masked row" result (computed in
numpy from column means) into the masked positions.

Math notes (same as the dense predecessor):
 - Softmax normalization is deferred: LN is invariant to positive row scale,
   so we feed LN with y'' = r2*y1 - (lam*r1)*y2 where r1/r2 are exp-row-sums.
 - The trailing (1 - lambda_init) factor is folded into ln_w / ln_b.
 - The 1/sqrt(HS) score scale is folded into Wq host-side.
 - Partition-dim reductions (row sums r, LN mean/var) are done with a
   ones[128,128] matmul on the tensor engine instead of gpsimd.
"""

import math
import os
import sys

sys.path.insert(0, "/opt/trn_rl_repo")

# Engine-usage toggles.  The conservative defaults (all DMA on the SP HWDGE
# ring, epilogue element-wise ops on DVE rather than gpsimd) are both faster
# in the cost model and match the engine paths proven on this runtime.
SP_ONLY_DMA = os.environ.get("KOPT_ACTDMA") != "1"
NO_POOL = os.environ.get("KOPT_POOL") != "1"

import ml_dtypes
import numpy as np

import concourse.bass as bass
import concourse.bass_isa as bass_isa
import concourse.mybir as mybir
from concourse import bacc
from concourse.bass import ds, ts
from concourse.bass_utils import run_bass_kernel_spmd
from concourse.tile import TileContext

B, T, C, H = 2, 2048, 1024, 16
HS = C // H            # 64
D2 = 2 * HS            # 128
LAYER_IDX = 2
LAMBDA_INIT = 0.8 - 0.6 * float(np.exp(-0.3 * (LAYER_IDX - 1)))
EPS = 1e-9
N_CORES = 8
GPB = N_CORES // B          # core groups per batch = 4
HPC = H // GPB              # heads per core = 4
NKT = T // 128              # 16 k tiles
DEFAULT_NQA = 12            # 1046 active rows -> 1536 (rounded to 512-multiples)
DEFAULT_NQ_OUT = 9          # out-projection tiles actually read (ceil(1046/128))

FP32 = mybir.dt.float32
BF16 = mybir.dt.bfloat16
AF = mybir.ActivationFunctionType
ALU = mybir.AluOpType

_CACHED = {}


def _q_blocks(ta):
    """Split the active-q extent into blocks of <=512 (PSUM-bank-legal sizes).

    Matmul outputs may not cross a 2KB PSUM bank boundary, so the two score
    maps laid out side by side at [0:sz] and [sz:2sz] require sz in
    {128, 256, 512}: either both maps in bank 0 (sz<=256) or exactly
    bank-aligned (sz=512).
    """
    out, off = [], 0
    while off < ta:
        rem = ta - off
        sz = 512 if rem >= 512 else (256 if rem >= 256 else 128)
        out.append((off, sz))
        off += sz
    return out


def build_nc(repeat=1, mode='all', nqa=None, nq_out=None, nqa_comp=None):
    nqa = DEFAULT_NQA if nqa is None else nqa
    nq_out = DEFAULT_NQ_OUT if nq_out is None else min(nq_out, nqa)
    # Compute extent (q rows actually processed) can be smaller than the
    # tensor-shape extent TA: non-512-multiple TENSOR shapes fail at NEFF
    # execution, but restricting the block list to ceil(active/128)*128
    # rows while keeping 512-multiple shapes runs fine and skips the
    # padding work.
    nqa_comp = (nq_out if nqa_comp is None
                else min(nqa_comp, nqa))
    TA = nqa * 128
    TP = -(-TA // 512) * 512          # transpose/DMA extent, 512-multiple
    blocks = _q_blocks(min(nqa_comp, nqa) * 128)

    nc = bacc.Bacc("TRN2", target_bir_lowering=False, debug=False,
                   enable_asserts=False)

    xq_d = nc.dram_tensor("xq", [TP, C], BF16, kind="ExternalInput").ap()
    xk_d = nc.dram_tensor("xk", [T, C], BF16, kind="ExternalInput").ap()
    xv_d = nc.dram_tensor("xv", [T, C], BF16, kind="ExternalInput").ap()
    # weights, host packed to SBUF layout (partition dim first)
    wq_d = nc.dram_tensor("wq", [128, HPC * 8 * 128], BF16, kind="ExternalInput").ap()
    wk_d = nc.dram_tensor("wk", [128, HPC * 8 * 128], BF16, kind="ExternalInput").ap()
    wv_d = nc.dram_tensor("wv", [128, 8 * 512], BF16, kind="ExternalInput").ap()
    wc_d = nc.dram_tensor("wc", [128, HPC * 1024], BF16, kind="ExternalInput").ap()
    lnw_d = nc.dram_tensor("lnw", [128, 1], FP32, kind="ExternalInput").ap()
    lnb_d = nc.dram_tensor("lnb", [128, 1], FP32, kind="ExternalInput").ap()
    lq1_d = nc.dram_tensor("lq1", [1, HPC * HS], FP32, kind="ExternalInput").ap()
    lk1_d = nc.dram_tensor("lk1", [1, HPC * HS], FP32, kind="ExternalInput").ap()
    lq2_d = nc.dram_tensor("lq2", [1, HPC * HS], FP32, kind="ExternalInput").ap()
    lk2_d = nc.dram_tensor("lk2", [1, HPC * HS], FP32, kind="ExternalInput").ap()
    out_d = nc.dram_tensor("out", [TA, C], FP32, kind="ExternalOutput").ap()

    gp = nc.vector if NO_POOL else nc.gpsimd

    with TileContext(nc) as tc:
      for _rep in range(repeat):
        with (
            tc.tile_pool(name="singles", bufs=1) as singles,
            tc.tile_pool(name="maps", bufs=1) as maps,
        ):
            # ---------- constants / tiny prep ----------
            lnw_sb = singles.tile([128, 1], FP32, tag="lnw")
            lnb_sb = singles.tile([128, 1], FP32, tag="lnb")
            nc.sync.dma_start(out=lnw_sb, in_=lnw_d)
            nc.sync.dma_start(out=lnb_sb, in_=lnb_d)

            # lambda per head: lam = exp(sum(lq1*lk1)) - exp(sum(lq2*lk2)) + l0
            lrow = singles.tile([1, HPC * HS], FP32, tag="lrow")
            lrow2 = singles.tile([1, HPC * HS], FP32, tag="lrow2")
            ltmp = singles.tile([1, HPC * HS], FP32, tag="ltmp")
            s1 = singles.tile([1, HPC], FP32, tag="s1")
            s2 = singles.tile([1, HPC], FP32, tag="s2")
            lam_row = singles.tile([1, HPC], FP32, tag="lam_row")
            nc.sync.dma_start(out=lrow, in_=lq1_d)
            nc.sync.dma_start(out=lrow2, in_=lk1_d)
            nc.vector.tensor_mul(ltmp, lrow, lrow2)
            nc.vector.reduce_sum(s1, ltmp.rearrange("p (h d) -> p h d", d=HS),
                                 axis=mybir.AxisListType.X)
            nc.sync.dma_start(out=lrow, in_=lq2_d)
            nc.sync.dma_start(out=lrow2, in_=lk2_d)
            nc.vector.tensor_mul(ltmp, lrow, lrow2)
            nc.vector.reduce_sum(s2, ltmp.rearrange("p (h d) -> p h d", d=HS),
                                 axis=mybir.AxisListType.X)
            nc.scalar.activation(s1, s1, AF.Exp)
            nc.scalar.activation(s2, s2, AF.Exp)
            nc.vector.tensor_sub(lam_row, s1, s2)
            nc.vector.tensor_scalar_add(lam_row, lam_row, LAMBDA_INIT)
            lam_col = singles.tile([128, HPC], FP32, tag="lam_col")
            nc.gpsimd.partition_broadcast(lam_col, lam_row, 128)

            eps_col = singles.tile([128, 1], FP32, tag="eps_col")
            nc.vector.memset(eps_col, EPS)
            ones_sb = singles.tile([128, 128], BF16, tag="ones")
            nc.vector.memset(ones_sb, 1.0)

            wc_sb = singles.tile([128, HPC * 1024], BF16, tag="wc")
            nc.sync.dma_start(out=wc_sb, in_=wc_d)

            # ---------- transposed x loads + projections ----------
            qmapT = [maps.tile([128, TA], BF16, tag=f"qm{h}", name=f"qm{h}")
                     for h in range(HPC)]
            kmapT = [maps.tile([128, T], BF16, tag=f"km{h}", name=f"km{h}")
                     for h in range(HPC)]
            vv = [maps.tile([128, 4 * D2], BF16, tag=f"vv{i}", name=f"vv{i}")
                  for i in range(NKT)]
            ynormT = [maps.tile([128, TA], BF16, tag=f"yn{h}", name=f"yn{h}")
                      for h in range(HPC)]

            with (
                tc.tile_pool(name="wpool", bufs=1) as wpool,
                tc.tile_pool(name="xt", bufs=10) as xt_pool,
                tc.tile_pool(name="ppsum", bufs=4, space="PSUM") as ppsum,
            ):
                wq_sb = wpool.tile([128, HPC * 8 * 128], BF16, tag="wq")
                wk_sb = wpool.tile([128, HPC * 8 * 128], BF16, tag="wk")
                wv_sb = wpool.tile([128, 8 * 512], BF16, tag="wv")
                nc.sync.dma_start(out=wv_sb, in_=wv_d)
                nc.sync.dma_start(out=wk_sb, in_=wk_d)
                nc.sync.dma_start(out=wq_sb, in_=wq_d)

                def w_qk(w_sb, h, ct):   # [128, 128] lhsT (C-tile ct, head h)
                    return w_sb[:, ds((h * 8 + ct) * 128, 128)]

                def load_xt(x_d, nm, cols):
                    # Chunk each transpose into 512-row pieces so consumers
                    # start as soon as their rows land, and alternate the
                    # SP / Activation HWDGE queues so the two DMA rings run
                    # in parallel on hardware.
                    tiles = [xt_pool.tile([128, T], BF16, tag="xt",
                                          name=f"{nm}{i}") for i in range(8)]
                    for c0 in range(0, cols, 512):
                        cw = min(512, cols - c0)
                        for i in range(8):
                            eng = nc.sync if (SP_ONLY_DMA or i % 2 == 0) \
                                else nc.scalar
                            eng.dma_start_transpose(
                                tiles[i][:, ds(c0, cw)],
                                x_d[ds(c0, cw), ds(i * 128, 128)])
                    return tiles

                xvT = load_xt(xv_d, "xv", T)
                for kt in range(NKT):
                    ps = ppsum.tile([128, 512], FP32, tag="ppsum", name="pp")
                    for ct in range(8):
                        nc.tensor.matmul(ps, xvT[ct][:, ds(kt * 128, 128)],
                                         wv_sb[:, ds(ct * 512, 512)],
                                         start=(ct == 0), stop=(ct == 7))
                    nc.vector.tensor_copy(vv[kt], ps)

                xkT = load_xt(xk_d, "xk", T)
                for h in range(HPC):
                    for kb in range(T // 512):
                        ps = ppsum.tile([128, 512], FP32, tag="ppsum", name="pp")
                        for ct in range(8):
                            nc.tensor.matmul(ps, w_qk(wk_sb, h, ct),
                                             xkT[ct][:, ds(kb * 512, 512)],
                                             start=(ct == 0), stop=(ct == 7))
                        nc.scalar.activation(kmapT[h][:, ds(kb * 512, 512)],
                                             ps, AF.Copy)

                xqT = load_xt(xq_d, "xq", TP)
                for h in range(HPC):
                    for (off, sz) in blocks:
                        ps = ppsum.tile([128, 512], FP32, tag="ppsum", name="pp")
                        for ct in range(8):
                            nc.tensor.matmul(ps[:, 0:sz], w_qk(wq_sb, h, ct),
                                             xqT[ct][:, ds(off, sz)],
                                             start=(ct == 0), stop=(ct == 7))
                        nc.scalar.activation(qmapT[h][:, ds(off, sz)],
                                             ps[:, 0:sz], AF.Copy)

            # ---------- attention ----------
            # Blocks are lists of "lanes" (h, qoff, sz, coff): full 512-wide
            # blocks have one lane; the <=128 remainder tiles of all 4 heads
            # are fused into ONE tail block (4 lanes) so the kt loop keeps a
            # uniform ~1us rhythm instead of running overhead-bound on tiny
            # per-head tail blocks.  Column layout within the shared [128,1024]
            # psum/sbuf tiles: lane maps sit at [coff, coff+sz) and
            # [c2off, c2off+sz) with c2off chosen so no matmul output crosses
            # a 2KB PSUM bank.
            # Lane tuples are (h, off, sz, coff, c2): the two score maps of a
            # lane live at psum cols [coff, coff+sz) and [c2, c2+sz).  Matmul
            # outputs must START at a 2KB psum bank boundary (col 0 or 512 of
            # the [128, 1024] tile) on this runtime -- sub-bank output offsets
            # fail at NEFF execution.  So every non-fused lane uses coff=0 /
            # c2=512 regardless of sz; for sz<512 the cols [sz, 512) are
            # simply unused.
            full_per_head = [(off, sz) for (off, sz) in blocks if sz == 512]
            tail = [(off, sz) for (off, sz) in blocks if sz != 512]
            blk_list = []
            for h in range(HPC):
                for (off, sz) in full_per_head:
                    blk_list.append([(h, off, sz, 0, 512)])
            if tail:
                for h in range(HPC):
                    for (off, sz) in tail:
                        blk_list.append([(h, off, sz, 0, 512)])

            def lane_cols(lanes):
                tot = 0
                for (h, off, sz, coff, c2) in lanes:
                    tot = max(tot, c2 + sz)
                return tot

            with (
                tc.tile_pool(name="escr", bufs=6) as e_pool,
                tc.tile_pool(name="scr", bufs=2) as scr_pool,
                tc.tile_pool(name="spsum", bufs=2, space="PSUM") as spsum,
                tc.tile_pool(name="ypsum", bufs=1, space="PSUM") as ypsum,
                tc.tile_pool(name="epsum", bufs=1, space="PSUM") as epsum,
            ):
                def regions(sz):
                    # (start, width) spans of the two maps inside a [128,1024]
                    # tile; merged into one span when contiguous (sz=512).
                    if sz == 512:
                        return [(0, 1024)]
                    return [(0, sz), (512, sz)]

                def yregions(sz):
                    # per-map matmul output spans (may not cross a psum bank)
                    return [(0, sz), (512, sz)]

                def emit_s(lane, kt):
                    (h, off, sz, coff, c2) = lane[0]
                    ksl = ds(kt * 128, 128)
                    qsl = ds(off, sz)
                    s = spsum.tile([128, 2 * 512], FP32, tag="s", name="s")
                    nc.tensor.matmul(s[:, ds(coff, sz)],
                                     kmapT[h][0:64, ksl],
                                     qmapT[h][0:64, qsl],
                                     start=True, stop=True,
                                     tile_position=(0, 0))
                    nc.tensor.matmul(s[:, ds(c2, sz)],
                                     kmapT[h][64:128, ksl],
                                     qmapT[h][64:128, qsl],
                                     start=True, stop=True,
                                     tile_position=(64, 0))
                    return s

                def emit_deferred_a(p):
                    ylnq, sz = p["ylnq"], p["sz"]
                    stats = epsum.tile([128, 2 * 512], FP32, tag="ep",
                                       name="stats")
                    for (c0, cw) in yregions(sz):
                        nc.tensor.matmul(stats[:, ds(c0, cw)], ones_sb,
                                         ylnq[:, ds(c0, cw)],
                                         start=True, stop=True)
                    mv = scr_pool.tile([128, 2 * 512], FP32, tag="mv",
                                       name="mv")
                    for (c0, cw) in regions(sz):
                        nc.vector.tensor_scalar(mv[:, ds(c0, cw)],
                                                stats[:, ds(c0, cw)],
                                                1.0 / D2, None, op0=ALU.mult)
                    (h, off, sz, coff, c2) = p["lane"][0]
                    mean = mv[:, ds(coff, sz)]
                    var = mv[:, ds(c2, sz)]
                    msq = scr_pool.tile([128, 512], FP32, tag="msq",
                                        name="msq")[:, 0:sz]
                    gp.tensor_mul(msq, mean, mean)
                    gp.tensor_sub(var, var, msq)
                    p["mv"] = mv

                def emit_deferred_b(p):
                    mv = p["mv"]
                    (h, off, sz, coff, c2) = p["lane"][0]
                    qsl = ds(off, sz)
                    yln = p["ylnq"][:, ds(coff, sz)]
                    mean = mv[:, ds(coff, sz)]
                    var = mv[:, ds(c2, sz)]
                    # rstd = exp(-0.5 * ln(var + eps))
                    nc.scalar.activation(var, var, AF.Ln, bias=eps_col)
                    nc.scalar.activation(var, var, AF.Exp, scale=-0.5)
                    tno = scr_pool.tile([128, 512], FP32, tag="tno",
                                        name="tno")[:, 0:sz]
                    gp.tensor_sub(tno, yln, mean)
                    gp.tensor_mul(tno, tno, var)
                    nc.vector.tensor_scalar(ynormT[h][:, qsl], tno,
                                            lnw_sb, lnb_sb,
                                            op0=ALU.mult, op1=ALU.add)

                pend = None
                s_carry = None
                for bi, lane in enumerate(blk_list):
                    (h, off, sz, coff, c2) = lane[0]
                    hsl = ds(h * D2, D2)
                    y12 = ypsum.tile([128, 2 * 512], FP32, tag="y", name="y12")
                    ra0 = scr_pool.tile([128, 2 * 512], BF16, tag="ra0", name="ra0")
                    ra1 = scr_pool.tile([128, 2 * 512], BF16, tag="ra1", name="ra1")
                    s_next = s_carry if s_carry is not None else emit_s(lane, 0)
                    s_carry = None
                    for kt in range(NKT):
                        s = s_next
                        if kt + 1 < NKT:
                            s_next = emit_s(lane, kt + 1)
                        elif bi + 1 < len(blk_list):
                            s_carry = emit_s(blk_list[bi + 1], 0)
                        if kt < 2:
                            e = (ra0 if kt == 0 else ra1)
                        else:
                            e = e_pool.tile([128, 2 * 512], BF16,
                                            tag="e", name="e")
                        for (c0, cw) in regions(sz):
                            nc.scalar.activation(e[:, ds(c0, cw)],
                                                 s[:, ds(c0, cw)], AF.Exp)
                        for (c0, cw) in yregions(sz):
                            nc.tensor.matmul(y12[:, ds(c0, cw)],
                                             vv[kt][:, hsl], e[:, ds(c0, cw)],
                                             start=(kt == 0),
                                             stop=(kt == NKT - 1))
                        if kt >= 2:
                            tgt = (ra0 if kt % 2 == 0 else ra1)
                            for (c0, cw) in regions(sz):
                                nc.vector.tensor_add(tgt[:, ds(c0, cw)],
                                                     tgt[:, ds(c0, cw)],
                                                     e[:, ds(c0, cw)])
                        if kt == 4 and pend is not None:
                            emit_deferred_a(pend)
                        if kt == 10 and pend is not None:
                            emit_deferred_b(pend)
                            pend = None

                    # ---- quick epilogue: drain PSUM, prep LN inputs ----
                    rsum = scr_pool.tile([128, 2 * 512], BF16, tag="rsum", name="rsum")
                    for (c0, cw) in regions(sz):
                        nc.vector.tensor_add(rsum[:, ds(c0, cw)],
                                             ra0[:, ds(c0, cw)],
                                             ra1[:, ds(c0, cw)])
                    rall = epsum.tile([128, 2 * 512], FP32, tag="ep",
                                      name="rall")
                    for (c0, cw) in yregions(sz):
                        nc.tensor.matmul(rall[:, ds(c0, cw)], ones_sb,
                                         rsum[:, ds(c0, cw)],
                                         start=True, stop=True)
                    ylnq = scr_pool.tile([128, 2 * 512], BF16, tag="ylnq", name="ylnq")
                    r1 = rall[:, ds(coff, sz)]
                    r2 = rall[:, ds(c2, sz)]
                    y1h = scr_pool.tile([128, 512], FP32, tag="y1h", name="y1h")[:, 0:sz]
                    nc.vector.tensor_copy(y1h, y12[:, ds(coff, sz)])
                    c2t = scr_pool.tile([128, 512], FP32, tag="c2", name="c2")[:, 0:sz]
                    nc.vector.tensor_scalar(c2t, r1, lam_col[:, ds(h, 1)],
                                            None, op0=ALU.mult)
                    nc.vector.tensor_mul(y1h, y1h, r2)
                    nc.vector.tensor_mul(c2t, c2t, y12[:, ds(c2, sz)])
                    yln = ylnq[:, ds(coff, sz)]
                    ysq = ylnq[:, ds(c2, sz)]
                    nc.vector.tensor_sub(yln, y1h, c2t)
                    nc.vector.tensor_mul(ysq, yln, yln)
                    pend = dict(lane=lane, ylnq=ylnq, sz=sz)
                emit_deferred_a(pend)
                emit_deferred_b(pend)

            # ---------- output projection ----------
            with (
                tc.tile_pool(name="obuf", bufs=2) as ob_pool,
                tc.tile_pool(name="opsum", bufs=4, space="PSUM") as opsum,
            ):
                for qt_i in range(nq_out):
                    qsl = ds(qt_i * 128, 128)
                    ob = ob_pool.tile([128, C], FP32, tag="ob", name="ob")
                    for cs in range(2):
                        ps = opsum.tile([128, 512], FP32, tag="op", name="op")
                        for h in range(HPC):
                            nc.tensor.matmul(ps, ynormT[h][:, qsl],
                                             wc_sb[:, ds(h * 1024 + cs * 512, 512)],
                                             start=(h == 0), stop=(h == HPC - 1))
                        nc.vector.tensor_copy(ob[:, ds(cs * 512, 512)], ps)
                    nc.sync.dma_start(out=out_d[qsl, :], in_=ob)

    # Force every activation (Exp + Ln + Copy + Square) onto the combined
    # natural_log_exp_and_others table set so no ACT_TABLE_LOADs are emitted
    # mid-kernel (~2.7us per switch).
    _orig_tables = bacc.get_activation_tables

    def _only_combined(arch):
        out = {}
        for name, funcs in _orig_tables(arch).items():
            out[name] = funcs if name == "natural_log_exp_and_others" else set()
        return out

    bacc.get_activation_tables = _only_combined
    try:
        nc.compile()
    finally:
        bacc.get_activation_tables = _orig_tables
    return nc


def _nqa_for(mask):
    # Round the active-row extent up to a multiple of 512 (4 tiles): the
    # 512-wide q-blocks are the only shapes validated end-to-end on this
    # runtime (non-512-multiple extents fail at NEFF execution).
    counts = (np.asarray(mask) != 0).sum(axis=1)
    nqa = max(1, int(math.ceil(counts.max() / 128.0)))
    return min(T // 128, -(-nqa // 4) * 4)


def _active_sel(mask_b, ta):
    """Active row indices first, padded with masked rows up to ta."""
    act = np.nonzero(mask_b != 0)[0]
    msk = np.nonzero(mask_b == 0)[0]
    pad = ta - len(act)
    if pad > 0:
        sel = np.concatenate([act, msk[:pad]])
    else:
        sel = act[:ta]
    return sel, min(len(act), ta)


def _prep_core_inputs(inputs, core, nqa=None):
    if nqa is None:
        nqa = int(os.environ.get("KOPT_NQA", "0")) or _nqa_for(inputs["mask"])
    ta = nqa * 128
    tp = -(-ta // 512) * 512
    b = core // GPB
    g = core % GPB
    h2 = slice(g * HPC * D2, (g + 1) * HPC * D2)          # 128/head cols
    bf = ml_dtypes.bfloat16
    sel, _ = _active_sel(np.asarray(inputs["mask"])[b], tp)

    qscale = np.float32(1.0 / math.sqrt(HS))

    def pack_qk(w1, w2, scale):
        # -> [128, HPC*8*128]: per head the 8 C-tiles of [W1_h | W2_h]
        cols = []
        for h in range(HPC):
            hh = slice((g * HPC + h) * HS, (g * HPC + h + 1) * HS)
            w = np.concatenate([w1[:, hh], w2[:, hh]], axis=1) * scale
            cols.append(w.reshape(8, 128, 128))
        arr = np.stack(cols, 0)                    # [HPC, 8, 128, 128]
        return np.ascontiguousarray(
            arr.transpose(2, 0, 1, 3).reshape(128, -1)).astype(bf)

    wv = inputs["Wv"][:, h2].reshape(8, 128, HPC * D2)
    wv = np.ascontiguousarray(wv.transpose(1, 0, 2).reshape(128, -1)).astype(bf)
    wc = inputs["Wc"][h2, :].reshape(HPC, 128, C)
    wc = np.ascontiguousarray(wc.transpose(1, 0, 2).reshape(128, -1)).astype(bf)

    sc = np.float32(1.0 - LAMBDA_INIT)
    heads = slice(g * HPC, (g + 1) * HPC)
    return {
        "xq": inputs["q"][b][sel].astype(bf),
        "xk": inputs["k"][b].astype(bf),
        "xv": inputs["v"][b].astype(bf),
        "wq": pack_qk(inputs["Wq1"], inputs["Wq2"], qscale),
        "wk": pack_qk(inputs["Wk1"], inputs["Wk2"], np.float32(1.0)),
        "wv": wv,
        "wc": wc,
        "lnw": (inputs["ln_w"] * sc).astype(np.float32).reshape(128, 1),
        "lnb": (inputs["ln_b"] * sc).astype(np.float32).reshape(128, 1),
        "lq1": inputs["lq1"][heads].astype(np.float32).reshape(1, -1),
        "lk1": inputs["lk1"][heads].astype(np.float32).reshape(1, -1),
        "lq2": inputs["lq2"][heads].astype(np.float32).reshape(1, -1),
        "lk2": inputs["lk2"][heads].astype(np.float32).reshape(1, -1),
    }


def _masked_row(inputs, b):
    """Output row (minus bc) for a masked query position: uniform attention."""
    f64 = np.float64
    lam = (np.exp((inputs["lq1"].astype(f64) * inputs["lk1"]).sum(-1))
           - np.exp((inputs["lq2"].astype(f64) * inputs["lk2"]).sum(-1))
           + LAMBDA_INIT)                                    # [H]
    vbar = inputs["v"][b].astype(f64).mean(0) @ inputs["Wv"].astype(f64) \
        + inputs["bv"].astype(f64)                           # [2C]
    yh = vbar.reshape(H, D2) * (1.0 - lam)[:, None]          # [H, 2HS]
    u = yh.mean(-1, keepdims=True)
    s = ((yh - u) ** 2).mean(-1, keepdims=True)
    yn = inputs["ln_w"].astype(f64) * (yh - u) / np.sqrt(s + 1e-12) \
        + inputs["ln_b"].astype(f64)
    yn = yn * (1.0 - LAMBDA_INIT)
    return (yn.reshape(2 * C) @ inputs["Wc"].astype(f64)).astype(np.float32)


def kernel(q, k, v, mask, Wq1, bq1, Wq2, bq2, Wk1, bk1, Wk2, bk2,
           Wv, bv, Wc, bc, ln_w, ln_b, lq1, lk1, lq2, lk2, **run_kw):
    inputs = dict(q=np.asarray(q), k=np.asarray(k), v=np.asarray(v),
                  mask=np.asarray(mask), Wq1=np.asarray(Wq1),
                  Wq2=np.asarray(Wq2), Wk1=np.asarray(Wk1), Wk2=np.asarray(Wk2),
                  Wv=np.asarray(Wv), bv=np.asarray(bv), Wc=np.asarray(Wc),
                  ln_w=np.asarray(ln_w), ln_b=np.asarray(ln_b),
                  lq1=np.asarray(lq1), lk1=np.asarray(lk1),
                  lq2=np.asarray(lq2), lk2=np.asarray(lk2))
    nqa = int(os.environ.get("KOPT_NQA", "0")) or _nqa_for(inputs["mask"])
    ta = nqa * 128
    counts = (inputs["mask"] != 0).sum(axis=1)
    nq_out = min(nqa, max(1, -(-int(counts.max()) // 128)))
    key = ("nc", nqa, nq_out)
    if key not in _CACHED:
        _CACHED[key] = build_nc(nqa=nqa, nq_out=nq_out)
    nc = _CACHED[key]
    in_maps = [_prep_core_inputs(inputs, c, nqa) for c in range(N_CORES)]
    res = run_bass_kernel_spmd(nc, in_maps, list(range(N_CORES)), **run_kw)
    _CACHED["last_results"] = res

    out = np.zeros((B, T, C), np.float32)
    for b in range(B):
        acc = np.zeros((ta, C), np.float32)
        for gcore in range(GPB):
            acc += np.asarray(
                res.results[b * GPB + gcore]["out"]).astype(np.float32)
        sel, nact = _active_sel(inputs["mask"][b], ta)
        out[b][sel[:nact]] = acc[:nact]
        mrow = _masked_row(inputs, b)
        out[b][inputs["mask"][b] == 0] = mrow[None, :]
    out += np.asarray(bc, np.float32)[None, None, :]
    return out

